# revision 27
# baseline (speedup 1.0000x reference)
"""Trainium2 Bass kernel for nn_AdaptiveMobiusLayer.

Strategy (pure data parallel over tokens, 8 NeuronCores):
  - Flatten x [4, 4096, 1024] -> [16384, 1024] tokens; core c takes 2048
    consecutive tokens (= batch b = c//2, seq half c%2).
  - Host transposes each shard to [1024 feats, 2048 tokens] so every matmul
    keeps features on partitions (weights are natural [K, M] lhsT stationary
    operands, activations are the moving operand; no on-device transposes).
  - The seq-mean for the global context needs the partner core's partial
    sum.  The 8 cores do NOT start in lockstep (launch/DMA skew of 5-25us
    run-to-run), so a single AllReduce stalls the early core by the full
    skew.  Instead FOUR tiny pairwise AllReduces ([128, 8] f32 each) are
    issued as each chunk's partial sums complete - the pipeline absorbs the
    skew and only the last collective's tail is exposed.
  - MLP matmuls run in fp8 DoubleRow (fp32 accumulation in PSUM); `out`
    stays bf16 in SBUF across all 3 cycles.
  - Sigmoids are computed as 0.5 + 0.5*tanh(x/2) with the affine factors
    folded into the coupling constants, so every activation (Gelu / Tanh)
    lives in the single `gelu_and_others` ACT table set.
  - The coupling scale/bias (c0, cmul) are cycle-invariant (gf depends only
    on x), broadcast to [128,1] once, and folded into one DVE tensor_scalar
    on the broadcast PSUM tile - no ACT-side coup/cb_sb ops.
  - Cycle-0 fp8 conversions of chunks 1-3 carry accum_out, producing the
    global-context partial sums for free; chunk 0 (fp8 shipped from host)
    gets eight explicit DVE reduces, emitted first (its data lands first).
  - All twist/cast elementwise work stays on the DVE: gpsimd shares its
    SBUF port with the DVE and concurrent gpsimd tensor ops inflate DVE
    tensor_tensor from ~413ns to ~900ns.
  - Weights are shipped as one consolidated DMA per tensor (DMA issue costs
    ~0.65us of sequencer time each); gc-net weights are queued behind the
    collective triggers so their transfers don't contend with the x burst.
  - In cycles 1-2 each chunk's update rides the NEXT chunk's mlp L1 hook:
    the broadcast matmul enters the in-order PE queue a full MLP layer
    after its tanh dependency started (no PE stall), and only the last
    chunk's update is exposed in the tail.
"""

import sys

sys.path.insert(0, "/opt/trn_rl_repo")

import numpy as np

B, S, DIM = 4, 4096, 1024
NCORES = 8
TOK = B * S // NCORES  # 2048 tokens per core
CHUNK = 512
NCHUNK = TOK // CHUNK  # 4
NUM_CYCLES = 3
BASE_COUPLING = 0.1

# feature-quarter twist:  out_new[t] = out[t] + sign[t] * c * out[(t+4) % 8]
# tiles 0..7 are 128-feature slabs; quarters = [t0 t1 | t2 t3 | t4 t5 | t6 t7]
TWIST_SIGN = [+1, +1, -1, -1, -1, -1, +1, +1]

REPLICA_GROUPS = [[0, 1], [2, 3], [4, 5], [6, 7]]

_CACHE = {}


def _build_graph():
    import concourse.bass as bass
    import concourse.bacc as bacc
    import concourse.tile as tile
    import concourse.mybir as mybir

    f32 = mybir.dt.float32
    bf16 = mybir.dt.bfloat16
    f8 = mybir.dt.float8e4
    AF = mybir.ActivationFunctionType
    ALU = mybir.AluOpType
    AX = mybir.AxisListType
    DR = mybir.MatmulPerfMode.DoubleRow

    nc = bacc.Bacc(
        "TRN2", target_bir_lowering=False, debug=False, num_devices=NCORES
    )

    # ---- DRAM parameters (per-core shard; layouts prepared on host) ----
    x_d = nc.declare_dram_parameter("x", [DIM, TOK], bf16, isOutput=False)
    # cycle-0's fp8 moving operand ships fully from host: the first L1
    # matmul waits only for a 0.5MB DMA, no DVE casts are needed in cycle 0,
    # and the global-context sums reduce the fp8 data directly (0.5MB/chunk
    # lands ~4x faster than the bf16 x, pulling the collective ~15us
    # earlier; the fp8 quantization adds ~2% error on gmean which feeds a
    # sigmoid gate - well inside the accuracy budget)
    x8_d = nc.declare_dram_parameter("x8", [DIM, TOK], f8, isOutput=False)
    # coupling-net weights in fp8 (DoubleRow 2x matmul mode), packed in the
    # SBUF tile layout [p, fo, s, j] == w[s*128+p, fo*128+j]
    w1_d = nc.declare_dram_parameter("cn_w1", [128, 8, 8, 128], f8, isOutput=False)
    w2_d = nc.declare_dram_parameter("cn_w2", [128, 4, 8, 128], f8, isOutput=False)
    w3_d = nc.declare_dram_parameter("cn_w3", [128, 2, 4, 128], f8, isOutput=False)
    w4_d = nc.declare_dram_parameter("cn_w4", [128, 2, 1], f8, isOutput=False)
    # all biases + scalars packed into one small tensor (single DMA):
    # cols 0-7 b1, 8-11 b2, 12-13 b3, 14-17 gb1, 18-19 gb2;
    # scalars replicated across all 128 partitions:
    # [:,20]=b4/2 [:,21]=gb3/2 [:,22]=adaptive_range
    cst_d = nc.declare_dram_parameter("consts", [128, 23], f32, isOutput=False)
    gw1_d = nc.declare_dram_parameter("gc_w1", [128, 8, 512], bf16, isOutput=False)
    gw2_d = nc.declare_dram_parameter("gc_w2", [128, 4, 256], bf16, isOutput=False)
    gw3_d = nc.declare_dram_parameter("gc_w3", [128, 2, 1], bf16, isOutput=False)
    out_d = nc.declare_dram_parameter("out", [DIM, TOK], bf16, isOutput=True)

    with tile.TileContext(nc) as tc:
        with (
            tc.tile_pool(name="const", bufs=1) as const,
            tc.tile_pool(name="xres", bufs=1) as xres,
            tc.tile_pool(name="work", bufs=2) as work,
            tc.tile_pool(name="psm", bufs=5, space="PSUM") as psm,
            tc.tile_pool(name="psx", bufs=3, space="PSUM") as psx,
            tc.tile_pool(name="dram", bufs=1, space="DRAM") as dram,
        ):
            # ---------------- input loads ----------------
            # fp8 x chunks (gate the first matmuls + global sums) first on
            # sync; bf16 x / later-layer weights follow (not needed until
            # ~20us in)
            pending_xb = [None] * NCHUNK
            for c in range(NCHUNK):
                x8c = work.tile([128, 8, CHUNK], f8, tag=f"x8_{c}",
                                name=f"x8_{c}")
                nc.sync.dma_start(
                    out=x8c[:], in_=x8_d[:, c * CHUNK:(c + 1) * CHUNK]
                    .rearrange("(s p) j -> p s j", p=128))
                pending_xb[c] = x8c

            # coupling-net L1 weights + consts on the scalar queue (ACT is
            # idle until the first GELU); consolidated DMAs - each dma_start
            # costs ~0.65us of sequencer issue time.  w1f before cst: the
            # first matmul needs w1f ~4us before the first GELU needs cst.
            w1f_all = const.tile([128, 8, 8, 128], f8, tag="w1f")
            # two halves so L1's first fo groups unblock a bit earlier
            nc.scalar.dma_start(out=w1f_all[:, 0:4], in_=w1_d[:, 0:4])
            nc.scalar.dma_start(out=w1f_all[:, 4:8], in_=w1_d[:, 4:8])
            w1f = [w1f_all[:, fo] for fo in range(8)]

            cst = const.tile([128, 23], f32, tag="cst")
            nc.scalar.dma_start(out=cst[:], in_=cst_d[:, :])
            b1 = cst[:, 0:8]
            b2 = cst[:, 8:12]
            b3 = cst[:, 12:14]
            gb1 = cst[:, 14:18]
            gb2 = cst[:, 18:20]
            b4h = cst[0:1, 20:21]
            gb3h = cst[0:1, 21:22]
            ar128 = cst[:, 22:23]
            ones = const.tile([1, 128], bf16, tag="ones")
            nc.vector.memset(ones[:], 1.0)

            w2f_all = const.tile([128, 4, 8, 128], f8, tag="w2f")
            nc.sync.dma_start(out=w2f_all[:], in_=w2_d[:, :, :, :])
            w2f = [w2f_all[:, fo] for fo in range(4)]
            w3f_all = const.tile([128, 2, 4, 128], f8, tag="w3f")
            nc.sync.dma_start(out=w3f_all[:], in_=w3_d[:, :, :, :])
            w3f = [w3f_all[:, fo] for fo in range(2)]
            w4f = const.tile([128, 2, 1], f8, tag="w4f")
            nc.sync.dma_start(out=w4f[:], in_=w4_d[:, :, :])

            # gc weights on sync after the fp8/weight burst: their 1.3MB
            # lands ~25us in, ahead of the ~54us gc hook, and the DMA rings
            # drain before the latency-critical cc_in feed (~30us)
            gw1_all = const.tile([128, 8, 512], bf16, tag="gw1")
            nc.sync.dma_start(out=gw1_all[:], in_=gw1_d[:, :, :])
            gw2_all = const.tile([128, 4, 256], bf16, tag="gw2")
            nc.sync.dma_start(out=gw2_all[:], in_=gw2_d[:, :, :])
            gw3_all = const.tile([128, 2, 1], bf16, tag="gw3")
            nc.sync.dma_start(out=gw3_all[:], in_=gw3_d[:, :, :])
            gw1 = [gw1_all[:, k] for k in range(8)]
            gw2 = [gw2_all[:, k] for k in range(4)]
            gw3 = [gw3_all[:, k] for k in range(2)]

            # bf16 x (residual/twist path): only needed by the cycle-0
            # updates (~55us in) - its 4MB is issued on the gpsimd queue
            # BEHIND the collective feed/trigger so those transfers never
            # sit on the DMA rings when the latency-critical 4KB cc_in runs
            out_bf = [[None] * NCHUNK for _ in range(8)]
            xchunk = [None] * NCHUNK

            def load_xc():
                for c in range(NCHUNK):
                    xc = xres.tile([128, 8, CHUNK], bf16, tag=f"xc_{c}",
                                   name=f"xc_{c}")
                    nc.gpsimd.dma_start(
                        out=xc[:], in_=x_d[:, c * CHUNK:(c + 1) * CHUNK]
                        .rearrange("(s p) j -> p s j", p=128))
                    xchunk[c] = xc
                    for t in range(8):
                        out_bf[t][c] = xc[:, t, :]

            out_q = [0]

            def dma_out_rr(out, in_):
                # rotate output DMAs across the sync/scalar/gpsimd queues so
                # the ~0.6us per-issue cost doesn't serialize the tail
                qs = (nc.sync, nc.scalar, nc.gpsimd)
                qs[out_q[0] % 3].dma_start(out=out, in_=in_)
                out_q[0] += 1

            # ---------------- global-context partial sums ----------------
            # per-(chunk, tile) partial sums, reduced straight off the fp8
            # x chunks on the DVE as each 0.5MB DMA lands
            red = const.tile([128, NCHUNK, 8], f32, tag="gred")

            def reduce_chunk(c):
                for t in range(8):
                    nc.vector.tensor_reduce(
                        red[:, c, t:t + 1], pending_xb[c][:, t, :],
                        axis=AX.X, op=ALU.add
                    )

            gs = const.tile([128, 8], f32, tag="gs")

            def finish_gsum():
                for t in range(8):
                    nc.vector.tensor_reduce(
                        gs[:, t:t + 1], red[:, :, t], axis=AX.X, op=ALU.add
                    )

            cc_in = dram.tile([128, 8], f32, tag="cc_in")
            cc_out = dram.tile([128, 8], f32, tag="cc_out")
            gmean_f = const.tile([128, 8], f32, tag="gmean_f")
            gmean = const.tile([128, 8], bf16, tag="gmean")

            def do_collective():
                # feed + trigger + return all on the otherwise-idle gpsimd
                # queue (the sync queue is busy streaming x/weights and would
                # delay the feed by ~20us).  The bf16 x loads are emitted
                # between trigger and return: their issue doesn't delay the
                # doorbell, and the return is gated on the collective anyway.
                nc.gpsimd.dma_start(out=cc_in[:], in_=gs[:])
                nc.gpsimd.collective_compute(
                    "AllReduce",
                    ALU.add,
                    ins=[cc_in.opt()],
                    outs=[cc_out.opt()],
                    replica_groups=REPLICA_GROUPS,
                )
                load_xc()
                nc.gpsimd.dma_start(out=gmean_f[:], in_=cc_out[:])
                nc.vector.tensor_copy(gmean[:], gmean_f[:])

            # ---------------- global net ----------------
            gc_state = {}

            def gc_all():
                # stage 1: all 4 output-tile groups accumulate into one PSUM
                # bank (disjoint columns) -> a single GELU epilogue
                ps = psx.tile([128, 4], f32, tag="aux")
                for fo in range(4):
                    for k in range(8):
                        nc.tensor.matmul(
                            ps[:, fo:fo + 1], gw1[k][:, fo * 128:(fo + 1) * 128],
                            gmean[:, k:k + 1], start=(k == 0), stop=(k == 7),
                        )
                # psum holds gc_w1.T @ sum(x); fold the 1/S mean + bias on DVE
                z1 = work.tile([128, 4], f32, tag="z1")
                nc.vector.scalar_tensor_tensor(
                    z1[:], ps[:], 1.0 / S, gb1, ALU.mult, ALU.add
                )
                g1 = work.tile([128, 4], bf16, tag="g1")
                nc.scalar.activation(g1[:], z1[:], AF.Gelu)
                # stage 2
                ps2 = psx.tile([128, 2], f32, tag="aux")
                for fo in range(2):
                    for k in range(4):
                        nc.tensor.matmul(
                            ps2[:, fo:fo + 1], gw2[k][:, fo * 128:(fo + 1) * 128],
                            g1[:, k:k + 1], start=(k == 0), stop=(k == 3),
                        )
                z2 = work.tile([128, 2], f32, tag="z2")
                nc.vector.tensor_add(z2[:], ps2[:], gb2)
                g2 = work.tile([128, 2], bf16, tag="g2")
                nc.scalar.activation(g2[:], z2[:], AF.Gelu)
                # stage 3
                ps3 = psx.tile([1, 1], f32, tag="aux")
                for k in range(2):
                    nc.tensor.matmul(
                        ps3[:], gw3[k][:, 0:1], g2[:, k:k + 1],
                        start=(k == 0), stop=(k == 1)
                    )
                # gf = sigmoid(z + gb3) = 0.5 + 0.5*tanh(z/2 + gb3/2); tanh
                # lives in the same ACT table set as Gelu (no table reload).
                tg = const.tile([1, 1], bf16, tag="tg")
                nc.scalar.activation(tg[:], ps3[:], AF.Tanh, bias=gb3h, scale=0.5)
                # broadcast tg across partitions (tiny K=1 matmul), then build
                # the cycle-invariant coupling constants as [128,1] vectors:
                #   coupling = c0 + cmul * t4
                #   cmul = 0.3*ar,  c0 = 0.1 + 0.7*ar*tg
                tgp = psx.tile([128, 1], f32, tag="aux")
                nc.tensor.matmul(tgp[:], ones[:], tg[:], start=True, stop=True)
                cmul = const.tile([128, 1], f32, tag="cmul")
                nc.vector.tensor_scalar(cmul[:], ar128, 0.3, None, ALU.mult)
                art = const.tile([128, 1], f32, tag="art")
                nc.vector.tensor_tensor(art[:], tgp[:], ar128, ALU.mult)
                c0 = const.tile([128, 1], f32, tag="c0")
                nc.vector.tensor_scalar(
                    c0[:], art[:], 0.7, BASE_COUPLING, ALU.mult, ALU.add)
                gc_state["cmul"] = cmul
                gc_state["c0"] = c0

            # ---------------- per-chunk building blocks ----------------
            def mlp_chunk(c, hooks=()):
                """coupling-net MLP on chunk c of `out`; returns the tf tile.

                hooks: up to 3 closures emitted after L1/L2/L3 - lets the
                gc-net chain and deferred updates ride the PE queue where
                each link's dependency has had a full layer's worth of
                matmuls to finish.
                """
                hooks = list(hooks) + [None] * 3
                xb = pending_xb[c]
                pending_xb[c] = None
                h1 = work.tile([128, 8, CHUNK], f8, tag="h1")
                for fo in range(8):
                    ps1 = psm.tile([128, CHUNK], f32, tag="mm")
                    for s in range(4):
                        nc.tensor.matmul(
                            ps1[:], w1f[fo][:, 2 * s:2 * s + 2, :],
                            xb[:, 2 * s:2 * s + 2, :],
                            start=(s == 0), stop=(s == 3), perf_mode=DR,
                        )
                    nc.scalar.activation(
                        h1[:, fo, :], ps1[:], AF.Gelu, bias=b1[:, fo:fo + 1])
                if hooks[0]:
                    hooks[0]()
                h2 = work.tile([128, 4, CHUNK], f8, tag="h2")
                for fo in range(4):
                    ps2 = psm.tile([128, CHUNK], f32, tag="mm")
                    for s in range(4):
                        nc.tensor.matmul(
                            ps2[:], w2f[fo][:, 2 * s:2 * s + 2, :],
                            h1[:, 2 * s:2 * s + 2, :],
                            start=(s == 0), stop=(s == 3), perf_mode=DR,
                        )
                    nc.scalar.activation(
                        h2[:, fo, :], ps2[:], AF.Gelu, bias=b2[:, fo:fo + 1])
                if hooks[1]:
                    hooks[1]()
                h3 = work.tile([128, 2, CHUNK], f8, tag="h3")
                for fo in range(2):
                    ps3 = psm.tile([128, CHUNK], f32, tag="mm")
                    for s in range(2):
                        nc.tensor.matmul(
                            ps3[:], w3f[fo][:, 2 * s:2 * s + 2, :],
                            h2[:, 2 * s:2 * s + 2, :],
                            start=(s == 0), stop=(s == 1), perf_mode=DR,
                        )
                    nc.scalar.activation(
                        h3[:, fo, :], ps3[:], AF.Gelu, bias=b3[:, fo:fo + 1])
                if hooks[2]:
                    hooks[2]()
                ps4 = psx.tile([1, CHUNK], f32, tag="aux")
                for s in range(2):
                    nc.tensor.matmul(
                        ps4[:], w4f[:, s, :], h3[:, s, :],
                        start=(s == 0), stop=(s == 1),
                    )
                # tf = sigmoid(z4+b4) -> carry t4 = tanh(z4/2 + b4/2) instead;
                # the 0.5/0.5 affine is folded into (c0, cmul).  bf16 so it
                # feeds the broadcast matmul directly.
                tf = work.tile([1, CHUNK], bf16, tag=f"tf_{c}")
                nc.scalar.activation(tf[:], ps4[:], AF.Tanh, bias=b4h, scale=0.5)
                return tf

            def update_chunk(c, tf, last, next_conv=False):
                """coupling + twist update (in place) on chunk c; DMA out if last."""
                # broadcast t4 across partitions via one bf16 K=1 matmul,
                # then apply the coupling scale/bias with a single DVE
                # tensor_scalar (scalar APs): cb = cmul*t4 + c0, cast to bf16
                # so every twist tensor_tensor op has pure bf16 operands.
                cb_ps = psx.tile([128, CHUNK], f32, tag="aux")
                nc.tensor.matmul(cb_ps[:], ones[:], tf[:], start=True, stop=True)
                cb_sb = work.tile([128, CHUNK], bf16, tag="cb_sb")
                nc.vector.tensor_scalar(
                    cb_sb[:], cb_ps[:], gc_state["cmul"][:], gc_state["c0"][:],
                    ALU.mult, ALU.add,
                )
                # twist update: pairs (t, t+4); all reads precede writes
                xb_next = None
                if next_conv and not last:
                    xb_next = work.tile([128, 8, CHUNK], f8, tag="xb", bufs=3)
                for p in range(4):
                    t, u = p, p + 4
                    tmpa = work.tile([128, CHUNK], bf16, tag="twa")
                    tmpb = work.tile([128, CHUNK], bf16, tag="twb")
                    nc.vector.tensor_mul(tmpa[:], out_bf[u][c], cb_sb[:])
                    nc.vector.tensor_mul(tmpb[:], out_bf[t][c], cb_sb[:])
                    if TWIST_SIGN[t] > 0:
                        nc.vector.tensor_add(out_bf[t][c], out_bf[t][c], tmpa[:])
                    else:
                        nc.vector.tensor_sub(out_bf[t][c], out_bf[t][c], tmpa[:])
                    if TWIST_SIGN[u] > 0:
                        nc.vector.tensor_add(out_bf[u][c], out_bf[u][c], tmpb[:])
                    else:
                        nc.vector.tensor_sub(out_bf[u][c], out_bf[u][c], tmpb[:])
                    if last:
                        # slabs t and u are final: issue their 0.25MB DMAs
                        # immediately so the final transfer tail is short
                        for tt in (t, u):
                            dma_out_rr(
                                out_d[tt * 128:(tt + 1) * 128,
                                      c * CHUNK:(c + 1) * CHUNK],
                                xchunk[c][:, tt, :],
                            )
                    elif next_conv:
                        nc.vector.tensor_copy(xb_next[:, t, :], out_bf[t][c])
                        nc.vector.tensor_copy(xb_next[:, u, :], out_bf[u][c])
                if next_conv and not last:
                    pending_xb[c] = xb_next

            # ---------------- main cycles ----------------
            # Cycle 0: the 32 partial-sum reduces run on the DVE as each fp8
            # chunk lands; one pairwise AllReduce as soon as they finish
            # (~28us) - its mesh completes during mlp(2)/mlp(3).  The gc-net
            # rides mlp(3)'s L1 hook; updates for chunks 0/1 ride the L2/L3
            # hooks.  (The gc DVE ops producing c0 must precede the update
            # tensor_scalars in the in-order DVE queue - updates may only be
            # emitted after gc_all.)
            tf0 = [None] * NCHUNK
            for c in range(NCHUNK):
                reduce_chunk(c)
            finish_gsum()
            do_collective()
            tf0[0] = mlp_chunk(0)
            tf0[1] = mlp_chunk(1)
            tf0[2] = mlp_chunk(2)
            tf0[3] = mlp_chunk(
                3,
                hooks=(
                    gc_all,
                    lambda: update_chunk(0, tf0[0], last=False, next_conv=True),
                    lambda: update_chunk(1, tf0[1], last=False, next_conv=True),
                ),
            )
            update_chunk(2, tf0[2], last=False, next_conv=True)
            update_chunk(3, tf0[3], last=False, next_conv=True)
            # Cycles 1-2: chunk c's update rides mlp(c+1)'s L1 hook - the
            # coupling broadcast matmul enters the in-order PE queue a full
            # MLP layer after its tanh dependency started, so the PE never
            # stalls at chunk boundaries; only chunk 3's update trails the
            # last mlp.
            for cyc in range(1, NUM_CYCLES):
                last = cyc == NUM_CYCLES - 1
                tfs = [None] * NCHUNK
                for c in range(NCHUNK):
                    hooks = ()
                    if c >= 1:
                        cc = c - 1
                        hooks = (
                            (lambda cc=cc: update_chunk(
                                cc, tfs[cc], last, next_conv=not last)),
                        )
                    tfs[c] = mlp_chunk(c, hooks=hooks)
                update_chunk(NCHUNK - 1, tfs[NCHUNK - 1], last,
                             next_conv=not last)

    nc.compile()
    return nc


def _get_graph():
    if "nc" not in _CACHE:
        _CACHE["nc"] = _build_graph()
    return _CACHE["nc"]


def _pack_consts(inputs):
    cst = np.zeros((128, 23), np.float32)
    cst[:, 0:8] = np.asarray(inputs["cn_b1"], np.float32).reshape(8, 128).T
    cst[:, 8:12] = np.asarray(inputs["cn_b2"], np.float32).reshape(4, 128).T
    cst[:, 12:14] = np.asarray(inputs["cn_b3"], np.float32).reshape(2, 128).T
    cst[:, 14:18] = np.asarray(inputs["gc_b1"], np.float32).reshape(4, 128).T
    cst[:, 18:20] = np.asarray(inputs["gc_b2"], np.float32).reshape(2, 128).T
    cst[:, 20] = 0.5 * float(np.asarray(inputs["cn_b4"]).reshape(()))
    cst[:, 21] = 0.5 * float(np.asarray(inputs["gc_b3"]).reshape(()))
    cst[:, 22] = float(np.asarray(inputs["adaptive_range"]).reshape(()))
    return cst


def _make_in_maps(inputs):
    import ml_dtypes

    bf = ml_dtypes.bfloat16
    f8 = ml_dtypes.float8_e4m3
    x = np.ascontiguousarray(inputs["x"], dtype=np.float32)
    xs32 = x.reshape(NCORES, TOK, DIM).transpose(0, 2, 1)  # [8, 1024, 2048]
    xs = xs32.astype(bf)

    shared = {
        # [p, fo, s, j] == w[s*128+p, fo*128+j]
        "cn_w1": np.ascontiguousarray(
            np.asarray(inputs["cn_w1"]).reshape(8, 128, 8, 128)
            .transpose(1, 2, 0, 3), dtype=f8),
        "cn_w2": np.ascontiguousarray(
            np.asarray(inputs["cn_w2"]).reshape(8, 128, 4, 128)
            .transpose(1, 2, 0, 3), dtype=f8),
        "cn_w3": np.ascontiguousarray(
            np.asarray(inputs["cn_w3"]).reshape(4, 128, 2, 128)
            .transpose(1, 2, 0, 3), dtype=f8),
        "cn_w4": np.ascontiguousarray(
            np.asarray(inputs["cn_w4"]).reshape(2, 128).T.reshape(128, 2, 1),
            dtype=f8),
        "gc_w1": np.ascontiguousarray(
            np.asarray(inputs["gc_w1"]).reshape(8, 128, 512)
            .transpose(1, 0, 2), dtype=bf),
        "gc_w2": np.ascontiguousarray(
            np.asarray(inputs["gc_w2"]).reshape(4, 128, 256)
            .transpose(1, 0, 2), dtype=bf),
        "gc_w3": np.ascontiguousarray(
            np.asarray(inputs["gc_w3"]).reshape(2, 128, 1)
            .transpose(1, 0, 2), dtype=bf),
        "consts": _pack_consts(inputs),
    }
    in_maps = []
    for c in range(NCORES):
        m = dict(shared)
        m["x"] = np.ascontiguousarray(xs[c])
        m["x8"] = np.ascontiguousarray(xs32[c]).astype(f8)
        in_maps.append(m)
    return in_maps


def _run(inputs, trace=False):
    from concourse.bass_utils import run_bass_kernel_spmd

    nc = _get_graph()
    in_maps = _make_in_maps(inputs)
    res = run_bass_kernel_spmd(
        nc, in_maps, core_ids=list(range(NCORES)), trace=trace
    )
    outs = np.stack(
        [np.asarray(res.results[c]["out"]).astype(np.float32).T
         for c in range(NCORES)], axis=0
    )  # [8, 2048, 1024]
    full = outs.reshape(B, S, DIM).astype(np.float32)
    return full, res


def kernel(**inputs) -> np.ndarray:
    out, _ = _run(inputs, trace=False)
    return out
